# revision 41
# baseline (speedup 1.0000x reference)
"""Causal scaled-dot-product attention on 8 Trainium2 NeuronCores.

Problem: B=2, H=16, S=2048, D=64, fp32, causal mask.
Sharding: batch*heads (32) split 4-per-core across 8 cores; no collectives.

v2 design (per core, 4 heads as 2 pairs):

Phase 1 (per k-chunk row ci, pair-parallel):
  - S^T[k, q] = (K^T)^T @ Q^T on PE (fp16); head A on PE rows 0-63,
    head B on rows 64-127 -> the two matmuls run concurrently.
  - P^T = exp(scale * S^T): pieces are split between ScalarE (exact
    spline exp) and VectorE (Schraudolph bit-trick: one tensor_scalar
    computing int16(x*A+B) whose bits ARE the fp16 exp) to halve the
    activation bottleneck. Causally packed into persistent SBUF fp16.
  - Diagonal 128x128 tile masked by GPSIMD affine_select.

Phase 2 (chain per (head, 128-col q-tile qt)): O^T[d,q] accumulated in
  PSUM [65, 128] with V_aug = [V | 1] STATIONARY (65-col LDW instead of
  128) and P^T chunks moving; the 65th row accumulates l = sum(P).
  Chain (h, qt) runs during mm1 row qt; its diagonal step runs one row
  later, so the post-mm1 tail is only 4 matmuls. No on-device
  normalization: PSUM is evacuated as fp16 (ScalarE/VectorE copy,
  load-balanced) and O = O_unnorm / l happens on the host.

Input DMA is fine-grained (per-piece completion semaphores), ordered by
first use, and spread across the sync + scalar HWDGE queues and gpsimd
SWDGE. mm2 chain steps (which depend only on older rows) are interleaved
between mm1 pieces so the PE never stalls on the exp engines.
"""

import sys
import numpy as np
from contextlib import ExitStack

B, H, S, D = 2, 16, 2048, 64
N_CORES = 8
HEADS_PER_CORE = (B * H) // N_CORES  # 4
CH = 128             # k-chunk (partition tile)
PIECE_W = 512        # S^T piece width per head (1 PSUM bank per head)
DP1 = D + 1
SCALE = 1.0 / np.sqrt(D)
LOG2E = 1.4426950408889634
# Schraudolph fp16 exp: bits = round(s_raw * A + B); bitcast fp16 ~= exp(s_raw/8)
SCH_A = float(SCALE * LOG2E * 1024.0)
SCH_B = float(15360.0 - 45.0)
# fraction of exp elements allowed on the DVE (bit-trick, ~1.8% rms noise,
# rel-err contribution ~= 1.6e-2 * sqrt(frac); keep well under the 2e-2 gate)
DVE_EXP_FRAC = 0.55

for _p in ("/opt/trn_rl_repo", "/opt/pypackages"):
    if _p not in sys.path:
        sys.path.append(_p)


def _row_off(ci, s_len):
    # packed column offset of causal row ci: sum_{j<ci} (s_len - 128*j)
    return s_len * ci - CH * (ci * (ci - 1)) // 2


def _build_program(n_heads=HEADS_PER_CORE, s_len=S, dve_frac=DVE_EXP_FRAC):
    import concourse.bass as bass  # noqa: F401
    import concourse.bacc as bacc
    import concourse.tile as tile
    from concourse import mybir

    f32 = mybir.dt.float32
    f16 = mybir.dt.float16
    i16 = mybir.dt.int16
    n_chunks = s_len // CH
    n_pairs = (n_heads + 1) // 2
    pt_len = _row_off(n_chunks, s_len)  # packed P^T length per head

    nc = bacc.Bacc(
        "TRN2",
        target_bir_lowering=False,
        debug=False,
        num_devices=N_CORES,
    )

    qk_d = nc.dram_tensor("qk", [128, n_pairs, 2, s_len], f16, kind="ExternalInput").ap()
    v_d = nc.dram_tensor("v", [128, n_chunks, n_heads, DP1], f16, kind="ExternalInput").ap()
    o_d = nc.dram_tensor("o", [n_heads, DP1, s_len], f16, kind="ExternalOutput").ap()

    # engine load-balancer state for exp pieces + psum evacuations
    eng_t = {"sc": 0.0, "ve": 0.0}
    exp_fd = {"ve": 0, "tot": 0}

    def pick_exp_engine(fd):
        cost_sc = 1.15 * (313.0 + fd) / 1.2
        cost_ve = (120.0 + fd) / 0.96
        use_ve = eng_t["ve"] + cost_ve < eng_t["sc"] + cost_sc
        if use_ve and (exp_fd["ve"] + fd) > dve_frac * (exp_fd["tot"] + fd):
            use_ve = False
        exp_fd["tot"] += fd
        if use_ve:
            exp_fd["ve"] += fd
            eng_t["ve"] += cost_ve
            return "ve"
        eng_t["sc"] += cost_sc
        return "sc"

    def pick_evac_engine(fd):
        cost_sc = 1.15 * (172.0 + fd) / 1.2
        cost_ve = (120.0 + fd) / 0.96
        if eng_t["ve"] + cost_ve < eng_t["sc"] + cost_sc:
            eng_t["ve"] += cost_ve
            return "ve"
        eng_t["sc"] += cost_sc
        return "sc"

    with tile.TileContext(nc) as tc, ExitStack() as ctx:
        const = ctx.enter_context(tc.tile_pool(name="const", bufs=1))
        sb_pt = ctx.enter_context(tc.tile_pool(name="ptp", bufs=n_pairs))
        sb_o = ctx.enter_context(tc.tile_pool(name="osb", bufs=1))
        ps_s = ctx.enter_context(tc.tile_pool(name="pss", bufs=1, space="PSUM"))
        ps_o = ctx.enter_context(tc.tile_pool(name="pso", bufs=1, space="PSUM"))

        qk = const.tile([128, n_pairs, 2, s_len], f16)
        v = const.tile([128, n_chunks, n_heads, DP1], f16)

        # --- input DMA: fine-grained, two HWDGE queues + SWDGE ---
        # sync queue: pair 0; scalar queue: pair 1. Row 0 needs only K
        # chunk 0 + the first Q piece, so those two go first on each
        # queue and the first matmul can start as soon as packets flow.
        # Per-row mm1 consumes only one new 32KB K chunk; Q (all columns)
        # is needed already for row 0. K chunk 0 and the Q pieces go
        # first; the rest of K follows.
        for pair, eng in zip(range(n_pairs), (nc.sync, nc.scalar)):
            eng.dma_start(out=qk[:, pair, 1, 0:2 * CH], in_=qk_d[:, pair, 1, 0:2 * CH])
            eng.dma_start(out=qk[:, pair, 0, 0:PIECE_W], in_=qk_d[:, pair, 0, 0:PIECE_W])
            eng.dma_start(out=qk[:, pair, 0, PIECE_W:2 * PIECE_W],
                          in_=qk_d[:, pair, 0, PIECE_W:2 * PIECE_W])
            eng.dma_start(out=qk[:, pair, 1, 2 * CH:8 * CH], in_=qk_d[:, pair, 1, 2 * CH:8 * CH])
            eng.dma_start(out=qk[:, pair, 0, 2 * PIECE_W:3 * PIECE_W],
                          in_=qk_d[:, pair, 0, 2 * PIECE_W:3 * PIECE_W])
            eng.dma_start(out=qk[:, pair, 0, 3 * PIECE_W:s_len],
                          in_=qk_d[:, pair, 0, 3 * PIECE_W:s_len])
            eng.dma_start(out=qk[:, pair, 1, 8 * CH:s_len],
                          in_=qk_d[:, pair, 1, 8 * CH:s_len])
        # v chunk-major: early chains need only low chunks of every head
        nc.gpsimd.dma_start(out=v[:, 0:4], in_=v_d[:, 0:4])
        nc.gpsimd.dma_start(out=v[:, 4:n_chunks], in_=v_d[:, 4:n_chunks])

        pts = {p: sb_pt.tile([128, 2, pt_len], f16, tag="ptfull", name=f"ptp{p}")
               for p in range(n_pairs)}
        stage = sb_o.tile([128, n_heads, s_len], f16, name="ostage")
        # O^T accumulators: 8 slots of [65, 128] f32 = 2 PSUM banks; slot
        # parity by qt keeps PE writes and ScalarE/DVE evac reads on
        # different banks.
        opb = ps_o.tile([DP1, 8, CH], f32, name="opbig")

        pair_heads = {p: [hh for hh in (2 * p, 2 * p + 1) if hh < n_heads]
                      for p in range(n_pairs)}

        def mm1_unit(pair, ci, poffs, big):
            """1-2 consecutive mm1 pieces sharing one exp op. A 2-piece
            unit needs the big (4-bank) tile and both pieces full-width;
            big/small tiles alternate globally so each has an implicit
            double buffer against the other."""
            heads = pair_heads[pair]
            pt_pair = pts[pair]
            sp0 = CH * ci
            span = s_len - sp0
            ro = _row_off(ci, s_len)
            if big:
                st = ps_s.tile([128, 2, 2, PIECE_W], f32, tag="stb", bufs=1,
                               name="stb")
            else:
                st = ps_s.tile([128, 1, 2, PIECE_W], f32, tag="st", bufs=1,
                               name="st")
            ws = [min(PIECE_W, span - poff) for poff in poffs]
            for pc, (poff, w) in enumerate(zip(poffs, ws)):
                for idx, hh in enumerate(heads):
                    bp = 64 * (hh % 2)
                    nc.tensor.matmul(
                        st[:, pc, idx, 0:w],
                        qk[bp:bp + 64, pair, 1, sp0:sp0 + CH],
                        qk[bp:bp + 64, pair, 0, sp0 + poff:sp0 + poff + w],
                        start=True,
                        stop=True,
                    )
            npc = len(poffs)
            p0 = poffs[0]
            if npc == 2:
                # both pieces full width: rectangular (pc, head, col) APs
                out_ap = pt_pair[:, 0:2, ro + p0:ro + p0 + 2 * PIECE_W] \
                    .rearrange("p h (c w) -> p c h w", c=2)
                in_ap = st[:, 0:2, 0:2, :]
                fd = 2 * 2 * PIECE_W
            else:
                out_ap = pt_pair[:, 0:2, ro + p0:ro + p0 + ws[0]]
                in_ap = st[:, 0, 0:2, 0:ws[0]]
                fd = 2 * ws[0]
            if pick_exp_engine(fd) == "sc":
                nc.scalar.activation(
                    out_ap, in_ap,
                    mybir.ActivationFunctionType.Exp,
                    scale=float(SCALE),
                )
            else:
                nc.vector.tensor_scalar(
                    out=out_ap.bitcast(i16), in0=in_ap,
                    scalar1=SCH_A, scalar2=SCH_B,
                    op0=mybir.AluOpType.mult, op1=mybir.AluOpType.add,
                )
            if p0 == 0:
                for idx in range(2):
                    nc.gpsimd.affine_select(
                        out=pt_pair[:, idx, ro:ro + CH],
                        in_=pt_pair[:, idx, ro:ro + CH],
                        compare_op=mybir.AluOpType.is_ge,
                        fill=0.0,
                        base=0,
                        pattern=[[1, CH]],
                        channel_multiplier=-1,
                    )

        def slot(hh, qt):
            return hh + n_heads * (qt % 2)

        def chain_step(hh, qt, ci2):
            """accumulation step ci2 of chain (hh, qt).

            start=True clears has_written for the WHOLE psum bank, so only
            the first chain of each qt-generation (hh==0) may set it; the
            other heads' first writes overwrite via the cleared bits.
            """
            pair, idx = divmod(hh, 2)
            sl = _row_off(ci2, s_len) + CH * (qt - ci2)
            nc.tensor.matmul(
                opb[:, slot(hh, qt), :],
                v[:, ci2, hh, :],
                pts[pair][:, idx, sl:sl + CH],
                start=(ci2 == 0 and hh == 0),
                stop=False,
                skip_group_check=True,
            )

        def diag_evac(qt):
            """final (diagonal) step + psum evacuation + output DMA."""
            for hh in range(n_heads):
                pair, idx = divmod(hh, 2)
                sl = _row_off(qt, s_len)
                nc.tensor.matmul(
                    opb[:, slot(hh, qt), :],
                    v[:, qt, hh, :],
                    pts[pair][:, idx, sl:sl + CH],
                    start=(qt == 0 and hh == 0),
                    stop=True,
                    skip_group_check=True,
                )
            # single merged evacuation of all 4 heads' accumulators
            s0 = n_heads * (qt % 2)
            src = opb[:, s0:s0 + n_heads, :]
            dst = stage[0:DP1, 0:n_heads, CH * qt:CH * (qt + 1)]
            if pick_evac_engine(n_heads * CH) == "sc":
                nc.scalar.copy(dst, src)
            else:
                nc.vector.tensor_copy(out=dst, in_=src)
            if (qt + 1) % 4 == 0:
                q0, q1 = CH * (qt - 3), CH * (qt + 1)
                for hh in range(n_heads):
                    eng = nc.sync if hh % 2 == 0 else nc.gpsimd
                    eng.dma_start(
                        out=o_d[hh][:, q0:q1], in_=stage[0:DP1, hh, q0:q1]
                    )

        # Interleave mm2 chain steps (which depend only on rows < ci, i.e.
        # always runnable) between mm1 units so the PE never stalls on the
        # exp engines when the score-tile rotation fills up.
        tag_toggle = [True]
        for ci in range(n_chunks):
            span = s_len - CH * ci
            queues = [list(range(0, span, PIECE_W)) for _ in range(n_pairs)]
            p_units = []
            pair_rr = 0
            while any(queues):
                pair = pair_rr if queues[pair_rr] else 1 - pair_rr
                qn = queues[pair]
                big = tag_toggle[0]
                tag_toggle[0] = not big
                take = 2 if (big and len(qn) >= 2
                             and qn[1] + PIECE_W <= span) else 1
                p_units.append((pair, qn[:take], big))
                del qn[:take]
                pair_rr = 1 - pair
            c_units = [(hh, ci, ci2)
                       for hh in range(n_heads)
                       for ci2 in range(ci)]
            if ci >= 1:
                diag_evac(ci - 1)
            k = 0
            for j, (pair, poffs, big) in enumerate(p_units):
                mm1_unit(pair, ci, poffs, big)
                tgt = (j + 1) * len(c_units) // len(p_units)
                while k < tgt:
                    chain_step(*c_units[k])
                    k += 1
            while k < len(c_units):
                chain_step(*c_units[k])
                k += 1
        diag_evac(n_chunks - 1)

    nc.compile()
    return nc


_PROGRAM_CACHE = {}


def _get_program(n_heads=HEADS_PER_CORE, s_len=S, dve_frac=DVE_EXP_FRAC):
    key = (n_heads, s_len, dve_frac)
    if key not in _PROGRAM_CACHE:
        _PROGRAM_CACHE[key] = _build_program(n_heads, s_len, dve_frac)
    return _PROGRAM_CACHE[key]


def _pack_core(Qf, Kf, Vf, heads, s_len=S):
    """Build the per-core input dict. Qf/Kf/Vf: [B*H, S, D] float32."""
    n_heads = len(heads)
    n_pairs = (n_heads + 1) // 2
    n_chunks = s_len // CH
    qk = np.zeros((128, n_pairs, 2, s_len), np.float16)
    v = np.ones((128, n_chunks, n_heads, DP1), np.float16)
    for i, hf in enumerate(heads):
        pair, side = divmod(i, 2)
        bp = 64 * side
        qk[bp:bp + 64, pair, 0] = Qf[hf].T
        qk[bp:bp + 64, pair, 1] = Kf[hf].T
        v[:, :, i, :D] = Vf[hf].reshape(n_chunks, CH, D).transpose(1, 0, 2)
    return {"qk": qk, "v": v}


def _unpack_core(o_np, s_len=S):
    """o_np: [n_heads, 65, S] fp16 (O^T unnormalized, l in row 64)
    -> [n_heads, S, D] f32 normalized."""
    o = o_np.astype(np.float32)
    out = o[:, :D, :] / o[:, D:D + 1, :]
    return out.transpose(0, 2, 1)


def kernel(Q, K, V, mask):
    Q = np.asarray(Q, np.float32)
    K = np.asarray(K, np.float32)
    V = np.asarray(V, np.float32)
    mask = np.asarray(mask)

    if not np.array_equal(mask, np.tril(np.ones((S, S), dtype=bool))):
        # Non-causal mask: not expected for this problem; numpy fallback.
        scores = np.einsum("bhqd,bhkd->bhqk", Q, K) * SCALE
        scores = np.where(mask, scores, -np.inf)
        scores -= scores.max(-1, keepdims=True)
        p = np.exp(scores)
        p /= p.sum(-1, keepdims=True)
        return np.einsum("bhqk,bhkd->bhqd", p, V).astype(np.float32)

    from concourse.bass_utils import run_bass_kernel_spmd

    Qf = Q.reshape(B * H, S, D)
    Kf = K.reshape(B * H, S, D)
    Vf = V.reshape(B * H, S, D)

    nc = _get_program()
    in_maps = [
        _pack_core(Qf, Kf, Vf, list(range(c * HEADS_PER_CORE, (c + 1) * HEADS_PER_CORE)))
        for c in range(N_CORES)
    ]
    res = run_bass_kernel_spmd(nc, in_maps, core_ids=list(range(N_CORES)))
    out = np.empty((B * H, S, D), np.float32)
    for c in range(N_CORES):
        out[c * HEADS_PER_CORE:(c + 1) * HEADS_PER_CORE] = _unpack_core(res.results[c]["o"])
    return out.reshape(B, H, S, D)


# revision 42
# speedup vs baseline: 1.2373x; 1.2373x over previous
"""Causal scaled-dot-product attention on 8 Trainium2 NeuronCores.

Problem: B=2, H=16, S=2048, D=64, fp32, causal mask.
Sharding: batch*heads (32) split 4-per-core across 8 cores; no collectives.

v2 design (per core, 4 heads as 2 pairs):

Phase 1 (per k-chunk row ci, pair-parallel):
  - S^T[k, q] = (K^T)^T @ Q^T on PE (fp16); head A on PE rows 0-63,
    head B on rows 64-127 -> the two matmuls run concurrently.
  - P^T = exp(scale * S^T): pieces are split between ScalarE (exact
    spline exp) and VectorE (Schraudolph bit-trick: one tensor_scalar
    computing int16(x*A+B) whose bits ARE the fp16 exp) to halve the
    activation bottleneck. Causally packed into persistent SBUF fp16.
  - Diagonal 128x128 tile masked by GPSIMD affine_select.

Phase 2 (chain per (head, 128-col q-tile qt)): O^T[d,q] accumulated in
  PSUM [65, 128] with V_aug = [V | 1] STATIONARY (65-col LDW instead of
  128) and P^T chunks moving; the 65th row accumulates l = sum(P).
  Chain (h, qt) runs during mm1 row qt; its diagonal step runs one row
  later, so the post-mm1 tail is only 4 matmuls. No on-device
  normalization: PSUM is evacuated as fp16 (ScalarE/VectorE copy,
  load-balanced) and O = O_unnorm / l happens on the host.

Input DMA is fine-grained (per-piece completion semaphores), ordered by
first use, and spread across the sync + scalar HWDGE queues and gpsimd
SWDGE. mm2 chain steps (which depend only on older rows) are interleaved
between mm1 pieces so the PE never stalls on the exp engines.
"""

import sys
import numpy as np
from contextlib import ExitStack

B, H, S, D = 2, 16, 2048, 64
N_CORES = 8
HEADS_PER_CORE = (B * H) // N_CORES  # 4
CH = 128             # k-chunk (partition tile)
PIECE_W = 512        # S^T piece width per head (1 PSUM bank per head)
DP1 = D + 1
SCALE = 1.0 / np.sqrt(D)
LOG2E = 1.4426950408889634
# Schraudolph fp16 exp: bits = round(s_raw * A + B); bitcast fp16 ~= exp(s_raw/8)
SCH_A = float(SCALE * LOG2E * 1024.0)
SCH_B = float(15360.0 - 45.0)
# fraction of exp elements allowed on the DVE (bit-trick, ~1.8% rms noise,
# rel-err contribution ~= 1.6e-2 * sqrt(frac); keep well under the 2e-2 gate)
DVE_EXP_FRAC = 0.55

for _p in ("/opt/trn_rl_repo", "/opt/pypackages"):
    if _p not in sys.path:
        sys.path.append(_p)


def _row_off(ci, s_len):
    # packed column offset of causal row ci: sum_{j<ci} (s_len - 128*j)
    return s_len * ci - CH * (ci * (ci - 1)) // 2


def _build_program(n_heads=HEADS_PER_CORE, s_len=S, dve_frac=DVE_EXP_FRAC):
    import concourse.bass as bass  # noqa: F401
    import concourse.bacc as bacc
    import concourse.tile as tile
    from concourse import mybir

    f32 = mybir.dt.float32
    f16 = mybir.dt.float16
    i16 = mybir.dt.int16
    n_chunks = s_len // CH
    n_pairs = (n_heads + 1) // 2
    pt_len = _row_off(n_chunks, s_len)  # packed P^T length per head

    nc = bacc.Bacc(
        "TRN2",
        target_bir_lowering=False,
        debug=False,
        num_devices=N_CORES,
    )

    qk_d = nc.dram_tensor("qk", [128, n_pairs, 2, s_len], f16, kind="ExternalInput").ap()
    v_d = nc.dram_tensor("v", [128, n_chunks, n_heads, DP1], f16, kind="ExternalInput").ap()
    o_d = nc.dram_tensor("o", [n_heads, DP1, s_len], f16, kind="ExternalOutput").ap()

    # engine load-balancer state for exp pieces + psum evacuations
    eng_t = {"sc": 0.0, "ve": 0.0}
    exp_fd = {"ve": 0, "tot": 0}

    def pick_exp_engine(fd):
        cost_sc = 1.15 * (313.0 + fd) / 1.2
        cost_ve = (120.0 + fd) / 0.96
        use_ve = eng_t["ve"] + cost_ve < eng_t["sc"] + cost_sc
        if use_ve and (exp_fd["ve"] + fd) > dve_frac * (exp_fd["tot"] + fd):
            use_ve = False
        exp_fd["tot"] += fd
        if use_ve:
            exp_fd["ve"] += fd
            eng_t["ve"] += cost_ve
            return "ve"
        eng_t["sc"] += cost_sc
        return "sc"

    def pick_evac_engine(fd):
        cost_sc = 1.15 * (172.0 + fd) / 1.2
        cost_ve = (120.0 + fd) / 0.96
        if eng_t["ve"] + cost_ve < eng_t["sc"] + cost_sc:
            eng_t["ve"] += cost_ve
            return "ve"
        eng_t["sc"] += cost_sc
        return "sc"

    with tile.TileContext(nc) as tc, ExitStack() as ctx:
        const = ctx.enter_context(tc.tile_pool(name="const", bufs=1))
        sb_pt = ctx.enter_context(tc.tile_pool(name="ptp", bufs=n_pairs))
        sb_o = ctx.enter_context(tc.tile_pool(name="osb", bufs=1))
        ps_s = ctx.enter_context(tc.tile_pool(name="pss", bufs=3, space="PSUM"))
        ps_o = ctx.enter_context(tc.tile_pool(name="pso", bufs=1, space="PSUM"))

        qk = const.tile([128, n_pairs, 2, s_len], f16)
        v = const.tile([128, n_chunks, n_heads, DP1], f16)

        # --- input DMA: fine-grained, two HWDGE queues + SWDGE ---
        # sync queue: pair 0; scalar queue: pair 1. Row 0 needs only K
        # chunk 0 + the first Q piece, so those two go first on each
        # queue and the first matmul can start as soon as packets flow.
        # Per-row mm1 consumes only one new 32KB K chunk; Q (all columns)
        # is needed already for row 0. K chunk 0 and the Q pieces go
        # first; the rest of K follows.
        for pair, eng in zip(range(n_pairs), (nc.sync, nc.scalar)):
            eng.dma_start(out=qk[:, pair, 1, 0:2 * CH], in_=qk_d[:, pair, 1, 0:2 * CH])
            eng.dma_start(out=qk[:, pair, 0, 0:PIECE_W], in_=qk_d[:, pair, 0, 0:PIECE_W])
            eng.dma_start(out=qk[:, pair, 0, PIECE_W:2 * PIECE_W],
                          in_=qk_d[:, pair, 0, PIECE_W:2 * PIECE_W])
            eng.dma_start(out=qk[:, pair, 1, 2 * CH:8 * CH], in_=qk_d[:, pair, 1, 2 * CH:8 * CH])
            eng.dma_start(out=qk[:, pair, 0, 2 * PIECE_W:3 * PIECE_W],
                          in_=qk_d[:, pair, 0, 2 * PIECE_W:3 * PIECE_W])
            eng.dma_start(out=qk[:, pair, 0, 3 * PIECE_W:s_len],
                          in_=qk_d[:, pair, 0, 3 * PIECE_W:s_len])
            eng.dma_start(out=qk[:, pair, 1, 8 * CH:s_len],
                          in_=qk_d[:, pair, 1, 8 * CH:s_len])
        # v chunk-major: early chains need only low chunks of every head
        nc.gpsimd.dma_start(out=v[:, 0:4], in_=v_d[:, 0:4])
        nc.gpsimd.dma_start(out=v[:, 4:n_chunks], in_=v_d[:, 4:n_chunks])

        pts = {p: sb_pt.tile([128, 2, pt_len], f16, tag="ptfull", name=f"ptp{p}")
               for p in range(n_pairs)}
        stage = sb_o.tile([128, n_heads, s_len], f16, name="ostage")
        # O^T accumulators: 8 slots of [65, 128] f32 = 2 PSUM banks; slot
        # parity by qt keeps PE writes and ScalarE/DVE evac reads on
        # different banks.
        opb = ps_o.tile([DP1, 8, CH], f32, name="opbig")

        pair_heads = {p: [hh for hh in (2 * p, 2 * p + 1) if hh < n_heads]
                      for p in range(n_pairs)}

        def mm1_piece(pair, ci, poff):
            heads = pair_heads[pair]
            pt_pair = pts[pair]
            sp0 = CH * ci
            span = s_len - sp0
            ro = _row_off(ci, s_len)
            w = min(PIECE_W, span - poff)
            st = ps_s.tile([128, 2, PIECE_W], f32, tag="st")
            for idx, hh in enumerate(heads):
                bp = 64 * (hh % 2)
                nc.tensor.matmul(
                    st[:, idx, 0:w],
                    qk[bp:bp + 64, pair, 1, sp0:sp0 + CH],
                    qk[bp:bp + 64, pair, 0, sp0 + poff:sp0 + poff + w],
                    start=True,
                    stop=True,
                )
            out_ap = pt_pair[:, 0:len(heads), ro + poff:ro + poff + w]
            in_ap = st[:, 0:len(heads), 0:w]
            if pick_exp_engine(len(heads) * w) == "sc":
                nc.scalar.activation(
                    out_ap, in_ap,
                    mybir.ActivationFunctionType.Exp,
                    scale=float(SCALE),
                )
            else:
                nc.vector.tensor_scalar(
                    out=out_ap.bitcast(i16), in0=in_ap,
                    scalar1=SCH_A, scalar2=SCH_B,
                    op0=mybir.AluOpType.mult, op1=mybir.AluOpType.add,
                )
            if poff == 0:
                for idx in range(len(heads)):
                    nc.gpsimd.affine_select(
                        out=pt_pair[:, idx, ro:ro + CH],
                        in_=pt_pair[:, idx, ro:ro + CH],
                        compare_op=mybir.AluOpType.is_ge,
                        fill=0.0,
                        base=0,
                        pattern=[[1, CH]],
                        channel_multiplier=-1,
                    )

        def slot(hh, qt):
            return hh + n_heads * (qt % 2)

        def chain_step(hh, qt, ci2):
            """accumulation step ci2 of chain (hh, qt).

            start=True clears has_written for the WHOLE psum bank, so only
            the first chain of each qt-generation (hh==0) may set it; the
            other heads' first writes overwrite via the cleared bits.
            """
            pair, idx = divmod(hh, 2)
            sl = _row_off(ci2, s_len) + CH * (qt - ci2)
            nc.tensor.matmul(
                opb[:, slot(hh, qt), :],
                v[:, ci2, hh, :],
                pts[pair][:, idx, sl:sl + CH],
                start=(ci2 == 0 and hh == 0),
                stop=False,
                skip_group_check=True,
            )

        def diag_evac(qt):
            """final (diagonal) step + psum evacuation + output DMA."""
            for hh in range(n_heads):
                pair, idx = divmod(hh, 2)
                sl = _row_off(qt, s_len)
                nc.tensor.matmul(
                    opb[:, slot(hh, qt), :],
                    v[:, qt, hh, :],
                    pts[pair][:, idx, sl:sl + CH],
                    start=(qt == 0 and hh == 0),
                    stop=True,
                    skip_group_check=True,
                )
            # single merged evacuation of all 4 heads' accumulators
            s0 = n_heads * (qt % 2)
            src = opb[:, s0:s0 + n_heads, :]
            dst = stage[0:DP1, 0:n_heads, CH * qt:CH * (qt + 1)]
            if pick_evac_engine(n_heads * CH) == "sc":
                nc.scalar.copy(dst, src)
            else:
                nc.vector.tensor_copy(out=dst, in_=src)
            if (qt + 1) % 4 == 0:
                q0, q1 = CH * (qt - 3), CH * (qt + 1)
                for hh in range(n_heads):
                    eng = nc.sync if hh % 2 == 0 else nc.gpsimd
                    eng.dma_start(
                        out=o_d[hh][:, q0:q1], in_=stage[0:DP1, hh, q0:q1]
                    )

        # Interleave mm2 chain steps (which depend only on rows < ci, i.e.
        # always runnable) between mm1 units so the PE never stalls on the
        # exp engines when the score-tile rotation fills up.
        for ci in range(n_chunks):
            span = s_len - CH * ci
            p_units = [(pair, poff)
                       for poff in range(0, span, PIECE_W)
                       for pair in range(n_pairs)]
            c_units = [(hh, ci, ci2)
                       for hh in range(n_heads)
                       for ci2 in range(ci)]
            if ci >= 1:
                diag_evac(ci - 1)
            k = 0
            for j, (pair, poff) in enumerate(p_units):
                mm1_piece(pair, ci, poff)
                tgt = (j + 1) * len(c_units) // len(p_units)
                while k < tgt:
                    chain_step(*c_units[k])
                    k += 1
            while k < len(c_units):
                chain_step(*c_units[k])
                k += 1
        diag_evac(n_chunks - 1)

    nc.compile()
    return nc


_PROGRAM_CACHE = {}


def _get_program(n_heads=HEADS_PER_CORE, s_len=S, dve_frac=DVE_EXP_FRAC):
    key = (n_heads, s_len, dve_frac)
    if key not in _PROGRAM_CACHE:
        _PROGRAM_CACHE[key] = _build_program(n_heads, s_len, dve_frac)
    return _PROGRAM_CACHE[key]


def _pack_core(Qf, Kf, Vf, heads, s_len=S):
    """Build the per-core input dict. Qf/Kf/Vf: [B*H, S, D] float32."""
    n_heads = len(heads)
    n_pairs = (n_heads + 1) // 2
    n_chunks = s_len // CH
    qk = np.zeros((128, n_pairs, 2, s_len), np.float16)
    v = np.ones((128, n_chunks, n_heads, DP1), np.float16)
    for i, hf in enumerate(heads):
        pair, side = divmod(i, 2)
        bp = 64 * side
        qk[bp:bp + 64, pair, 0] = Qf[hf].T
        qk[bp:bp + 64, pair, 1] = Kf[hf].T
        v[:, :, i, :D] = Vf[hf].reshape(n_chunks, CH, D).transpose(1, 0, 2)
    return {"qk": qk, "v": v}


def _unpack_core(o_np, s_len=S):
    """o_np: [n_heads, 65, S] fp16 (O^T unnormalized, l in row 64)
    -> [n_heads, S, D] f32 normalized."""
    o = o_np.astype(np.float32)
    out = o[:, :D, :] / o[:, D:D + 1, :]
    return out.transpose(0, 2, 1)


def kernel(Q, K, V, mask):
    Q = np.asarray(Q, np.float32)
    K = np.asarray(K, np.float32)
    V = np.asarray(V, np.float32)
    mask = np.asarray(mask)

    if not np.array_equal(mask, np.tril(np.ones((S, S), dtype=bool))):
        # Non-causal mask: not expected for this problem; numpy fallback.
        scores = np.einsum("bhqd,bhkd->bhqk", Q, K) * SCALE
        scores = np.where(mask, scores, -np.inf)
        scores -= scores.max(-1, keepdims=True)
        p = np.exp(scores)
        p /= p.sum(-1, keepdims=True)
        return np.einsum("bhqk,bhkd->bhqd", p, V).astype(np.float32)

    from concourse.bass_utils import run_bass_kernel_spmd

    Qf = Q.reshape(B * H, S, D)
    Kf = K.reshape(B * H, S, D)
    Vf = V.reshape(B * H, S, D)

    nc = _get_program()
    in_maps = [
        _pack_core(Qf, Kf, Vf, list(range(c * HEADS_PER_CORE, (c + 1) * HEADS_PER_CORE)))
        for c in range(N_CORES)
    ]
    res = run_bass_kernel_spmd(nc, in_maps, core_ids=list(range(N_CORES)))
    out = np.empty((B * H, S, D), np.float32)
    for c in range(N_CORES):
        out[c * HEADS_PER_CORE:(c + 1) * HEADS_PER_CORE] = _unpack_core(res.results[c]["o"])
    return out.reshape(B, H, S, D)
